# revision 4
# baseline (speedup 1.0000x reference)
"""Trainium2 Bass kernel for nn_CubicalModel_ISM.

Computes Xp = X @ p and Yp = Y @ p (X, Y: [784, 32768] f32, p: [32768] f32),
then gathers persistence-diagram values from the reshaped [28, 28] images.

Sharding: q (parameter) axis split across 8 NeuronCores, 4096 columns each.
Each core streams its [784, 4096] shards of X and Y through SBUF and does a
fused multiply + free-axis reduce on the Vector engine (tensor_tensor_reduce),
producing per-core partial row sums [784] per tensor. The [784] partials are
summed across cores on the host (tiny), and the 200-element gathers run on the
host as well.

Per-core on-chip layout: rows are processed 112 partitions at a time, 7 chunks
of [112, 4096] per tensor (112 * 7 = 784). chunk k, partition p = row
112*k + p. p-vector is DMA'd once to partition 0 and broadcast to all 112
partitions via GpSimd.
"""

import numpy as np

H = W = 28
Q = 32768
N_CORES = 8
QS = Q // N_CORES  # 4096 per-core q shard
R = H * W          # 784 rows
P = 112            # SBUF partitions used per chunk
RPP = R // P       # 7 chunks (rows-per-partition)

_CACHE = {}


def _build_nc():
    import concourse.bacc as bacc
    import concourse.mybir as mybir
    from concourse.tile import TileContext

    # Bacc (not raw Bass) is required: its compile() runs
    # generate_event_semaphores, which splits multi-wait instructions into
    # the 1-wait-per-instruction form this walrus accepts.
    nc = bacc.Bacc(None)
    f32 = mybir.dt.float32
    x = nc.dram_tensor("x", [R, QS], f32, kind="ExternalInput")
    y = nc.dram_tensor("y", [R, QS], f32, kind="ExternalInput")
    p = nc.dram_tensor("p", [1, QS], f32, kind="ExternalInput")
    out = nc.dram_tensor("out", [P, 2 * RPP], f32, kind="ExternalOutput")

    with TileContext(nc) as tc:
        with (
            tc.tile_pool(name="pbpool", bufs=1) as pb_pool,
            tc.tile_pool(name="chunks", bufs=4) as chunk_pool,
            tc.tile_pool(name="scratch", bufs=1) as scratch_pool,
            tc.tile_pool(name="respool", bufs=1) as res_pool,
        ):
            p_row = pb_pool.tile([1, QS], f32)
            pb = pb_pool.tile([P, QS], f32)
            nc.sync.dma_start(out=p_row[:, :], in_=p[:, :])
            nc.gpsimd.partition_broadcast(pb[:, :], p_row[:, :], channels=P)

            res = res_pool.tile([P, 2 * RPP], f32)
            scratch = scratch_pool.tile([P, QS], f32)
            for t, src in enumerate((x, y)):
                for k in range(RPP):
                    chunk = chunk_pool.tile([P, QS], f32)
                    nc.sync.dma_start(
                        out=chunk[:, :], in_=src[P * k : P * (k + 1), :]
                    )
                    col = t * RPP + k
                    # out = (chunk * 1.0) * pb elementwise (into scratch,
                    # discarded); accum_out = per-partition sum of out — the
                    # fused multiply + free-axis reduce in a single DVE pass.
                    nc.vector.scalar_tensor_tensor(
                        out=scratch[:, :],
                        in0=chunk[:, :],
                        scalar=1.0,
                        in1=pb[:, :],
                        op0=mybir.AluOpType.mult,
                        op1=mybir.AluOpType.mult,
                        accum_out=res[:, col : col + 1],
                    )
            nc.sync.dma_start(out=out[:, :], in_=res[:, :])
    nc.finalize()
    return nc


def _get_nc():
    if "nc" not in _CACHE:
        _CACHE["nc"] = _build_nc()
    return _CACHE["nc"]


def _run_on_cores(X, Y, p):
    from concourse.bass_utils import run_bass_kernel_spmd

    nc = _get_nc()
    in_maps = []
    for c in range(N_CORES):
        sl = slice(c * QS, (c + 1) * QS)
        in_maps.append(
            {
                "x": np.ascontiguousarray(X[:, sl]),
                "y": np.ascontiguousarray(Y[:, sl]),
                "p": np.ascontiguousarray(p[sl]).reshape(1, QS),
            }
        )
    return run_bass_kernel_spmd(nc, in_maps, list(range(N_CORES)))


def kernel(X, Y, p, inds1, inds2):
    X = np.asarray(X, dtype=np.float32)
    Y = np.asarray(Y, dtype=np.float32)
    p = np.asarray(p, dtype=np.float32)
    inds1 = np.asarray(inds1)
    inds2 = np.asarray(inds2)

    results = _run_on_cores(X, Y, p).results

    xp = np.zeros(R, dtype=np.float32)
    yp = np.zeros(R, dtype=np.float32)
    for c in range(N_CORES):
        o = results[c]["out"]  # [112, 14]; [p, k] = row 112*k + p
        xp += o[:, :RPP].T.reshape(R)
        yp += o[:, RPP:].T.reshape(R)

    def gather(img, inds):
        ij = inds.reshape(-1, 2)
        return img[ij[:, 0], ij[:, 1]].reshape(-1, 2)

    dgm1 = gather(xp.reshape(H, W), inds1)
    dgm2 = gather(yp.reshape(H, W), inds2)
    return dgm1, dgm2
